# revision 5
# baseline (speedup 1.0000x reference)
"""Bass/Trainium2 kernel v2 for nn_Attn (32,4096,512 attention pooling).

  energy = tanh(x @ W.T); ae = v . energy; w = softmax(ae over T)
  out[b] = sum_t w[b,t] * x[b,t,:]

Design (8 cores, data-parallel over B, 4 batches/core):
  - host sends TWO layouts of x:
      x8  [B_LOC, 128, 4, T]  fp8e4, x transposed (h-partitioned) for the
          energy matmul moving operand (halves that load's HBM traffic;
          an fp8 moving operand runs at bf16 speed on PE, and fp8 x with
          bf16 W keeps rel-err ~1.1e-2, well under the 2e-2 gate)
      xn  [B_LOC, 128, 32, H] f16, natural (t-partitioned) for the
          weighted sum
    both pre-arranged so each partition's bytes are contiguous.
  - all input DMAs ride the SP queue, chunked, with next-batch prefetch
    emitted ahead of the current batch's compute (a DMA holds its issuing
    engine's queue for the whole transfer, so big loads never share a
    queue with compute or with dependent small DMAs)
  - energy per 512-token block: per g-chunk 4 accumulating MMs
    (lhsT = W^T tile bf16 stationary, rhs = x8 fp8 moving) -> PSUM
  - tanh on ACT from PSUM (bf16 out)
  - v-dot: DVE tensor_scalar x4 (4x mode) + tensor_tensor tree x3 (2x),
    ones-matmul partition-reduce -> ae [1,512] PSUM -> f16 row wf.
    vred matmuls are emitted one block late so the PE FIFO never waits
    on the DVE chain.
  - ae relayout per half-batch: [1,2048] -> DRAM -> xbar transpose ->
    [128,16]; exp (bias=-8) on ACT with accum_out giving the softmax
    denominator partials for free
  - weighted sum: scalar_tensor_tensor chain on the otherwise-idle
    GPSIMD (fp32 acc); the LAST batch instead uses PE matmuls (PE is
    idle at the end, Pool would leave a long serial tail)
  - batch tails (partition-reduce, 1/S scale, store) are emitted one
    batch late so they never stall the PE FIFO
"""

import numpy as np
import ml_dtypes
from contextlib import ExitStack

import bass_rust
import concourse.bass as bass
import concourse.bass_isa as bass_isa
import concourse.mybir as mybir
import concourse.tile as tile
from concourse.bass_utils import run_bass_kernel_spmd

# ---------------------------------------------------------------------------
# Workaround: this container's walrus accepts only ONE sem-wait per
# instruction. Split extras onto same-engine NoOps before the instruction.


def _split_excess_waits(nc, max_waits=1):
    n_split = 0
    for fn in nc.m.functions:
        for blk in fn.blocks:
            new = []
            changed = False
            for inst in blk.instructions:
                si = inst.sync_info
                waits = list(si.on_wait) if si is not None else []
                if len(waits) > max_waits:
                    for w in waits[:-max_waits]:
                        nop = mybir.InstNoOp(
                            name=nc.get_next_instruction_name(),
                            engine=inst.engine,
                            ins=[],
                            outs=[],
                            sync_info=bass_rust.SyncInfo(
                                on_wait=[w], on_update=[]
                            ),
                        )
                        new.append(nop)
                        n_split += 1
                    inst.sync_info = bass_rust.SyncInfo(
                        on_wait=waits[-max_waits:], on_update=list(si.on_update)
                    )
                    changed = True
                new.append(inst)
            if changed:
                blk.instructions = new
    return n_split
# ---------------------------------------------------------------------------

B, T, H = 32, 4096, 512
N_CORES = 8
B_LOC = B // N_CORES
PC = 128
HC = H // PC                  # 4 h-chunks
GC = H // PC                  # 4 g-chunks
TBLK = 512
NBLK = T // TBLK              # 8 blocks per batch
NT = T // PC                  # 32 token subtiles per batch

BF16 = mybir.dt.bfloat16
F16 = mybir.dt.float16
F32 = mybir.dt.float32
FP8 = mybir.dt.float8e4
AF = mybir.ActivationFunctionType
MUL = mybir.AluOpType.mult
ADD = mybir.AluOpType.add

EXP_BIAS = -8.0
WSUM_ADD_ENGINE = "vector"    # "pool" | "vector" | "pe" (all batches on PE)
VRED_MODE = "cols"            # "cols" | "diag_row" (timing diagnostic)
VRED_LAG = 2                  # vred emission lag in blocks
WPROD_ACT_FRAC = 0            # every Nth wsum product on ACT (0 = none)
BUFS_TANH = 4
BUFS_ACC = 4
X8CHUNK = 8                   # xt8 load chunks per batch (one per block)
XNCHUNK = 4                   # xn load chunks per batch
TCH = T // X8CHUNK            # tokens per xt8 chunk
NCH = NT // XNCHUNK           # token subtiles per xn chunk


def set_xnchunk(n):
    global XNCHUNK, NCH
    XNCHUNK = n
    NCH = NT // n
EMIT_HALF0_AT = 5             # block index at which half 0's exp+chain is emitted


def _build_program(reps=1, split_waits=True):
    nc = bass.Bass()
    x8_d = nc.declare_dram_parameter("x8", [B_LOC, PC, HC, T], FP8, isOutput=False)
    xn_d = nc.declare_dram_parameter("xn", [B_LOC, PC, NT, H], F16, isOutput=False)
    wt_d = nc.declare_dram_parameter("wt", [PC, HC, H], BF16, isOutput=False)
    v_d = nc.declare_dram_parameter("v", [PC, GC], F32, isOutput=False)
    out_d = nc.declare_dram_parameter("out", [B_LOC, H], F32, isOutput=True)

    with tile.TileContext(nc) as tc, ExitStack() as ctx:
        singles = ctx.enter_context(tc.tile_pool(name="singles", bufs=1))
        x8p = ctx.enter_context(tc.tile_pool(name="x8p", bufs=2 * X8CHUNK))
        xnp = ctx.enter_context(tc.tile_pool(name="xnp", bufs=3 * XNCHUNK))
        tanhp = ctx.enter_context(tc.tile_pool(name="tanhp", bufs=BUFS_TANH))
        vprodp = ctx.enter_context(tc.tile_pool(name="vprodp", bufs=8))
        accp = ctx.enter_context(tc.tile_pool(name="accp", bufs=BUFS_ACC))
        wcp = ctx.enter_context(tc.tile_pool(name="wcp", bufs=2))
        oaccp = ctx.enter_context(tc.tile_pool(name="oaccp", bufs=3))
        wprodp = ctx.enter_context(tc.tile_pool(name="wprodp", bufs=6))
        wpairp = ctx.enter_context(tc.tile_pool(name="wpairp", bufs=4))
        sscp = ctx.enter_context(tc.tile_pool(name="sscp", bufs=2))
        smallp = ctx.enter_context(tc.tile_pool(name="smallp", bufs=4))
        osbp = ctx.enter_context(tc.tile_pool(name="osbp", bufs=2))
        dramp = ctx.enter_context(tc.tile_pool(name="dramp", bufs=2, space="DRAM"))
        pep = ctx.enter_context(tc.tile_pool(name="pep", bufs=2, space="PSUM"))
        paep = ctx.enter_context(tc.tile_pool(name="paep", bufs=2, space="PSUM"))
        pmixp = ctx.enter_context(tc.tile_pool(name="pmixp", bufs=2, space="PSUM"))

        wt_sb = singles.tile([PC, HC, H], BF16)
        nc.sync.dma_start(out=wt_sb, in_=wt_d[:])
        v_sb = singles.tile([PC, GC], F32)
        nc.sync.dma_start(out=v_sb, in_=v_d[:])
        ones16 = singles.tile([PC, 1], F16)
        nc.vector.memset(ones16, 1.0)
        ones32 = singles.tile([PC, 1], F32)
        nc.vector.memset(ones32, 1.0)
        ebias = singles.tile([PC, 1], F32)
        nc.vector.memset(ebias, EXP_BIAS)

        P = dict(
            tanhp=tanhp, vprodp=vprodp, accp=accp,
            wcp=wcp, oaccp=oaccp, wprodp=wprodp, wpairp=wpairp, sscp=sscp,
            smallp=smallp, osbp=osbp,
            dramp=dramp, pep=pep, paep=paep, pmixp=pmixp,
            wt_sb=wt_sb, v_sb=v_sb, ones16=ones16, ones32=ones32,
            ebias=ebias,
        )

        def _load_batch(b):
            """Chunked input DMAs for batch b, all on the SP queue.
            Separate tiles per chunk so consumers wait per-chunk."""
            x8c, xnc = [], []
            for c in range(X8CHUNK):
                t8 = x8p.tile([PC, HC, TCH], FP8, tag="xt8", name=f"xt8_{b}_{c}")
                nc.sync.dma_start(out=t8, in_=x8_d[b][:, :, c * TCH:(c + 1) * TCH])
                x8c.append(t8)
            for c in range(XNCHUNK):
                tn = xnp.tile([PC, NCH, H], F16, tag="xnat", name=f"xnat_{b}_{c}")
                nc.sync.dma_start(out=tn, in_=xn_d[b][:, c * NCH:(c + 1) * NCH, :])
                xnc.append(tn)
            return x8c, xnc

        def body():
            batches = [_Batch(nc, tc, b, out_d, P) for b in range(B_LOC)]
            loads = {0: _load_batch(0)}
            events = [(b, blk) for b in range(B_LOC) for blk in range(NBLK)]
            VL = VRED_LAG               # vred lag in blocks (global)
            for i, (b, blk) in enumerate(events):
                if blk == 0:
                    if b + 1 < B_LOC:
                        loads[b + 1] = _load_batch(b + 1)
                    batches[b].set_tiles(loads.pop(b))
                batches[b].energy(blk)
                if i >= VL:
                    b2, blk2 = events[i - VL]
                    batches[b2].vred(blk2)
                    if blk2 == NBLK // 2 - 1:
                        batches[b2].exp_and_chain(0)
                    elif blk2 == NBLK - 1:
                        batches[b2].exp_and_chain(1)
                if blk == NBLK - 1 and b >= 1:
                    batches[b - 1].finish()
            for i in range(len(events) - VL, len(events)):
                b2, blk2 = events[i]
                batches[b2].vred(blk2)
                if blk2 == NBLK - 1:
                    batches[b2].exp_and_chain(1)
            batches[B_LOC - 1].finish()

        if reps == 1:
            body()
        else:
            with tc.For_i(0, reps, 1):
                body()

    if split_waits:
        _split_excess_waits(nc)
    return nc


class _Batch:
    """Per-batch emission helpers; the body() event walk interleaves them."""

    def __init__(self, nc, tc, b, out_d, P):
        self.nc = nc
        self.b = b
        self.out_d = out_d
        self.P = P
        # "pe" mode: every batch's weighted sum on PE (f16 wc columns)
        self.last = (b == B_LOC - 1) or WSUM_ADD_ENGINE == "pe"
        self.wc = P["wcp"].tile([PC, NT], F16 if self.last else F32,
                                tag="wc", name=f"wc_{b}")
        self.ssc = P["sscp"].tile([PC, 2], F32, tag="ssc", name=f"ssc_{b}")
        self.pae = P["paep"].tile([PC, NT], F32, tag="pae", name=f"pae_{b}")
        self.acc = {}
        self.oacc = None
        self.po = None

    def set_tiles(self, tiles):
        self.x8c, self.xnc = tiles

    def energy(self, blk):
        nc, P, b = self.nc, self.P, self.b
        wt_sb = P["wt_sb"]; v_sb = P["v_sb"]
        xch = self.x8c[blk // (NBLK // X8CHUNK)]
        lt0 = (blk * TBLK) % TCH
        tEs = []
        for half in range(2):
            pe = P["pep"].tile([PC, 2, TBLK], F32, tag="pe",
                               name=f"pe_{b}_{blk}_{half}")
            for gl in range(2):
                gc = half * 2 + gl
                for hc in range(HC):
                    nc.tensor.matmul(
                        pe[:, gl, :],
                        lhsT=wt_sb[:, hc, gc * PC:(gc + 1) * PC],
                        rhs=xch[:, hc, lt0:lt0 + TBLK],
                        start=(hc == 0),
                        stop=(hc == HC - 1),
                    )
            tE = P["tanhp"].tile([PC, 2, TBLK], BF16, tag="tE",
                                 name=f"tE_{b}_{blk}_{half}")
            nc.scalar.activation(out=tE, in_=pe, func=AF.Tanh)
            tEs.append(tE)
        prods = []
        for gc in range(GC):
            p_ = P["vprodp"].tile([PC, TBLK], BF16, tag="vp",
                                  name=f"vp_{b}_{blk}_{gc}")
            nc.vector.tensor_scalar(out=p_, in0=tEs[gc // 2][:, gc % 2, :],
                                    scalar1=v_sb[:, gc:gc + 1], scalar2=None,
                                    op0=MUL)
            prods.append(p_)
        a01 = P["accp"].tile([PC, TBLK], BF16, tag="acc1", name=f"a01_{b}_{blk}")
        nc.vector.tensor_tensor(out=a01, in0=prods[0], in1=prods[1], op=ADD)
        a23 = P["accp"].tile([PC, TBLK], BF16, tag="acc1", name=f"a23_{b}_{blk}")
        nc.vector.tensor_tensor(out=a23, in0=prods[2], in1=prods[3], op=ADD)
        acc = P["accp"].tile([PC, TBLK], BF16, tag="acc2", name=f"acc_{b}_{blk}")
        nc.vector.tensor_tensor(out=acc, in0=a01, in1=a23, op=ADD)
        self.acc[blk] = acc

    def vred(self, blk):
        # ae columns: 4 single-column matmuls (lhsT = v-scaled tanh slice)
        nc, P = self.nc, self.P
        acc = self.acc.pop(blk)
        if VRED_MODE == "diag1":
            # TIMING DIAGNOSTIC ONLY (wrong results): 1 column per block
            nc.tensor.matmul(self.pae[:, blk * 4:blk * 4 + 1],
                             lhsT=acc[:, 0:PC], rhs=P["ones16"],
                             start=True, stop=True)
            return
        for jj in range(4):
            col = blk * 4 + jj
            nc.tensor.matmul(self.pae[:, col:col + 1],
                             lhsT=acc[:, jj * PC:(jj + 1) * PC],
                             rhs=P["ones16"], start=True, stop=True)

    def exp_and_chain(self, hb):
        nc, P, b = self.nc, self.P, self.b
        c0 = hb * (NT // 2)
        nc.scalar.activation(out=self.wc[:, c0:c0 + NT // 2],
                             in_=self.pae[:, c0:c0 + NT // 2], func=AF.Exp,
                             bias=P["ebias"], accum_out=self.ssc[:, hb:hb + 1])
        if self.last:
            if hb == 1:
                self._pe_wsum(0)
                self._pe_wsum(1)
            return
        # weighted sum: DVE makes w_j * x_j products (4x mode) and the
        # first pairwise-add level (2x mode); the otherwise-idle GPSIMD
        # runs the remaining serial accumulation adds
        for j2 in range(c0 // 2, (c0 + NT // 2) // 2):
            pair = []
            for j in (2 * j2, 2 * j2 + 1):
                xch = self.xnc[j // NCH]
                prod = P["wprodp"].tile([PC, H], F16, tag="wprod",
                                        name=f"wp_{b}_{j}")
                if WPROD_ACT_FRAC and j % WPROD_ACT_FRAC == 1:
                    nc.scalar.activation(out=prod, in_=xch[:, j % NCH, :],
                                         func=AF.Copy,
                                         scale=self.wc[:, j:j + 1])
                else:
                    nc.vector.tensor_scalar(out=prod, in0=xch[:, j % NCH, :],
                                            scalar1=self.wc[:, j:j + 1],
                                            scalar2=None, op0=MUL)
                pair.append(prod)
            ps = P["wpairp"].tile([PC, H], F16, tag="wpair", name=f"wq_{b}_{j2}")
            nc.vector.tensor_tensor(out=ps, in0=pair[0], in1=pair[1], op=ADD)
            if self.oacc is None:
                self.oacc = ps
                continue
            if WSUM_ADD_ENGINE == "pool":
                nxt = P["oaccp"].tile([PC, H], F32, tag="oacc",
                                      name=f"oacc_{b}_{j2}")
                nc.gpsimd.tensor_tensor(out=nxt, in0=ps, in1=self.oacc, op=ADD)
            else:
                nxt = P["oaccp"].tile([PC, H], F16, tag="oacc",
                                      name=f"oacc_{b}_{j2}")
                nc.vector.tensor_tensor(out=nxt, in0=ps, in1=self.oacc, op=ADD)
            self.oacc = nxt

    def _pe_wsum(self, hb):
        nc, P = self.nc, self.P
        if hb == 0:
            self.po = P["pmixp"].tile([1, H], F32, tag="po",
                                      name=f"pwsum_{self.b}")
        po = self.po
        for j in range(hb * NT // 2, (hb + 1) * NT // 2):
            nc.tensor.matmul(po, lhsT=self.wc[:, j:j + 1],
                             rhs=self.xnc[j // NCH][:, j % NCH, :],
                             start=(j == 0), stop=(j == NT - 1))

    def finish(self):
        # S: cross-partition sum of the exp weights.
        # b < last: tiny DRAM roundtrip of the exp accumulators
        #   ([128,2] -> DRAM -> [1,256] -> free-dim reduce) -- off the
        #   critical path mid-stream, avoids a PSUM bank.
        # last batch: ones-matmul over wc16 into a recycled pe-psum slot
        #   (all energy psum is free by then) -- keeps the loop-boundary
        #   SP tail short for the next repetition.
        nc, P, b = self.nc, self.P, self.b
        dma = nc.sync.dma_start
        if self.last:
            psS = P["pep"].tile([1, NT], F32, tag="pe", name=f"psS_{b}")
            nc.tensor.matmul(psS, lhsT=P["ones16"], rhs=self.wc,
                             start=True, stop=True)
            S = P["smallp"].tile([1, 1], F32, tag="S", name=f"S_{b}")
            nc.vector.tensor_reduce(out=S, in_=psS,
                                    axis=mybir.AxisListType.X, op=ADD)
        else:
            sdram = P["dramp"].tile([2, PC], F32, tag="sdram", name=f"sdram_{b}")
            dma(out=sdram, in_=self.ssc)
            srow = P["smallp"].tile([1, 2 * PC], F32, tag="srow", name=f"srow_{b}")
            dma(out=srow, in_=sdram)
            S = P["smallp"].tile([1, 1], F32, tag="S", name=f"S_{b}")
            nc.vector.tensor_reduce(out=S, in_=srow,
                                    axis=mybir.AxisListType.X, op=ADD)
        rS = P["smallp"].tile([1, 1], F32, tag="rS", name=f"rS_{b}")
        nc.vector.reciprocal(rS, S)
        if self.last:
            po = self.po
        else:
            po = P["pmixp"].tile([1, H], F32, tag="po", name=f"po_{b}")
            ones = P["ones32"] if WSUM_ADD_ENGINE == "pool" else P["ones16"]
            nc.tensor.matmul(po, lhsT=ones, rhs=self.oacc,
                             start=True, stop=True)
        ob = P["osbp"].tile([1, H], F32, tag="ob", name=f"ob_{b}")
        nc.vector.tensor_scalar(out=ob, in0=po, scalar1=rS, scalar2=None,
                                op0=MUL)
        dma(out=self.out_d[b:b + 1, :], in_=ob)


_PROGRAM = None


def _get_program():
    global _PROGRAM
    if _PROGRAM is None:
        _PROGRAM = _build_program()
    return _PROGRAM


def make_in_maps(x, W, v):
    """Host-side input prep: shard + relayout + cast. Not on the HW clock."""
    xt = np.ascontiguousarray(x.transpose(0, 2, 1))          # [B, H, T]
    x8 = xt.reshape(B, HC, PC, T).transpose(0, 2, 1, 3)      # [B, p, hc, T]
    x8 = np.ascontiguousarray(x8).astype(ml_dtypes.float8_e4m3)
    xn = x.reshape(B, NT, PC, H).transpose(0, 2, 1, 3)       # [B, p, nt, H]
    xn = np.ascontiguousarray(xn).astype(np.float16)
    wt = np.ascontiguousarray(W.T).reshape(HC, PC, H).transpose(1, 0, 2)
    wt = np.ascontiguousarray(wt).astype(ml_dtypes.bfloat16)  # [p, hc, g]
    vr = np.ascontiguousarray(v.reshape(GC, PC).T).astype(np.float32)
    return [
        {
            "x8": x8[c * B_LOC:(c + 1) * B_LOC],
            "xn": xn[c * B_LOC:(c + 1) * B_LOC],
            "wt": wt,
            "v": vr,
        }
        for c in range(N_CORES)
    ]


def run(inputs, trace=False, trace_kwargs=None):
    x = np.asarray(inputs["encoder_outputs"], dtype=np.float32)
    W = np.asarray(inputs["W"], dtype=np.float32)
    v = np.asarray(inputs["v"], dtype=np.float32)
    assert x.shape == (B, T, H)
    in_maps = make_in_maps(x, W, v)
    nc = _get_program()
    res = run_bass_kernel_spmd(
        nc, in_maps, list(range(N_CORES)), trace=trace, **(trace_kwargs or {}),
    )
    out = np.concatenate([res.results[c]["out"] for c in range(N_CORES)], axis=0)
    return out.astype(np.float32), res


def kernel(**inputs):
    out, _ = run(inputs)
    return out
